# revision 6
# baseline (speedup 1.0000x reference)
"""Attention kernel for trn2: B=4, N=2048, DIM=512, HEADS=8, DIM_HEAD=64.

Head-parallel across 8 cores (core h computes head h); host sums the 8
partial (bf16) outputs.

v5: the Activation engine runs exps only (the pipeline metronome); the
pos-bias multiply is folded in-place into the et tiles and split between
DVE and GPSIMD; PV is flipped to [q,65] orientation (half the PE cost)
and runs region-major one tile behind the exp stream; ho is re-oriented
for W_out via XBAR DMA transposes of region pairs; ebias streams through
SBUF in N/2-halves to make room for the 2-tile et working set.
"""

from collections import deque

import numpy as np
import ml_dtypes

B, N, DIM = 4, 2048, 512
HEADS, DH = 8, 64
P = 128
DC = DIM // P            # 4 contraction chunks of 128
NCH = N // P             # 16 n chunks of 128
NJ = N // 512            # 4 n chunks of 512
KC = N // P              # 16 k chunks
QT = 1024                # q tile in phase 2
NQT = N // QT            # 2
NR = QT // P             # 8 PV regions per tile

_CACHE = {}

# kc slots whose et-multiply runs on GPSIMD (Pool) instead of DVE
POOL_MUL_KCS = (1, 3, 5, 7, 9, 11, 13)


def _build():
    import concourse.mybir as mybir
    import concourse.tile as tile
    from concourse import bacc

    F32 = mybir.dt.float32
    BF16 = mybir.dt.bfloat16
    EXP = mybir.ActivationFunctionType.Exp
    MULT = mybir.AluOpType.mult
    BYP = mybir.AluOpType.bypass

    nc = bacc.Bacc(None, target_bir_lowering=False)

    xT_d = nc.dram_tensor("xT", [B, P, DC, N], BF16, kind="ExternalInput")
    wqk_d = nc.dram_tensor("wqk", [P, DC, P], BF16, kind="ExternalInput")
    wv_d = nc.dram_tensor("wv", [P, DC, DH], BF16, kind="ExternalInput")
    wout_d = nc.dram_tensor("wout", [DH, DIM], BF16, kind="ExternalInput")
    c4_d = nc.dram_tensor("c4", [P, N], BF16, kind="ExternalInput")
    s4_d = nc.dram_tensor("s4", [P, N], BF16, kind="ExternalInput")
    ebias_d = nc.dram_tensor("ebias", [P, KC, N], BF16, kind="ExternalInput")
    onesc_d = nc.dram_tensor("onesc", [P, KC], BF16, kind="ExternalInput")
    out_d = nc.dram_tensor("out", [B, NCH, P, DIM], BF16, kind="ExternalOutput")

    with tile.TileContext(nc) as tc:
        with tc.tile_pool(name="cp", bufs=1) as cp:
            wqk_t = cp.tile([P, DC, P], BF16, tag="wqk")
            nc.sync.dma_start(wqk_t[:], wqk_d[:, :, :])
            c4_t = cp.tile([P, N], BF16, tag="c4")
            nc.sync.dma_start(c4_t[:], c4_d[:, :])
            s4_t = cp.tile([P, N], BF16, tag="s4")
            nc.sync.dma_start(s4_t[:], s4_d[:, :])
            wv_t = cp.tile([P, DC, DH], BF16, tag="wv")
            wout_t = cp.tile([P, DIM], BF16, tag="wout")

            qT_b = [cp.tile([DH, N], BF16, tag=f"qT{b}", name=f"qT{b}")
                    for b in range(B)]
            kT_b = [cp.tile([DH, N], BF16, tag=f"kT{b}", name=f"kT{b}")
                    for b in range(B)]
            v_b = [cp.tile([P, KC, DH + 1], BF16, tag=f"v{b}", name=f"v{b}")
                   for b in range(B)]

            # ebias streamed in q-halves: [P, kc, 1024] for the current jq
            ebias_t = cp.tile([P, KC, QT], BF16, tag="ebias")

            def issue_x_dma(b):
                xt = cp.tile([P, DC, N], BF16, tag="xt", name=f"xt{b}", bufs=2)
                nc.sync.dma_start(xt[:, :, 0 : N // 2], xT_d[b, :, :, 0 : N // 2])
                nc.sync.dma_start(xt[:, :, N // 2 : N], xT_d[b, :, :, N // 2 : N])
                return xt

            alloc_ctx = {}

            def ph1_bundles(b, xt):
                """Phase-1 (QKV+rotary) work for batch b as callables.

                alloc_ctx["f"]() -> a [P, 512] F32 PSUM tile."""
                qk_sb = cp.tile([P, N], BF16, tag="qk_sb", name=f"qk_sb{b}", bufs=1)
                swap = cp.tile([P, N], BF16, tag="swap", name=f"swap{b}", bufs=1)
                t1 = cp.tile([P, N], BF16, tag="t1", name=f"t1_{b}", bufs=1)
                out = []

                def qk_chunk(j):
                    def f():
                        js = slice(j * 512, (j + 1) * 512)
                        qk_ps = alloc_ctx["f"]()
                        for dc in range(DC):
                            nc.tensor.matmul(
                                qk_ps[:],
                                lhsT=wqk_t[:, dc],
                                rhs=xt[:, dc, js],
                                start=(dc == 0),
                                stop=(dc == DC - 1),
                            )
                        nc.vector.tensor_copy(qk_sb[:, js], qk_ps[:])
                    return f

                def swaps():
                    nc.sync.dma_start(swap[0:32, :], qk_sb[32:64, :])
                    nc.sync.dma_start(swap[32:64, :], qk_sb[0:32, :])
                    nc.sync.dma_start(swap[64:96, :], qk_sb[96:128, :])
                    nc.sync.dma_start(swap[96:128, :], qk_sb[64:96, :])

                def rot_mul():
                    nc.vector.tensor_mul(t1[:], c4_t[:], qk_sb[:])
                    nc.vector.tensor_mul(swap[:], s4_t[:], swap[:])

                def rot_add():
                    nc.vector.tensor_add(qT_b[b][:], t1[0:DH, :], swap[0:DH, :])
                    nc.vector.tensor_add(kT_b[b][:], t1[DH:P, :], swap[DH:P, :])

                def v_group(g):
                    def f():
                        for i in range(g * 4, g * 4 + 4):
                            isl = slice(i * P, (i + 1) * P)
                            v_ps = alloc_ctx["f"]()
                            for dc in range(DC):
                                nc.tensor.matmul(
                                    v_ps[:, 0:DH],
                                    lhsT=xt[:, dc, isl],
                                    rhs=wv_t[:, dc],
                                    start=(dc == 0),
                                    stop=(dc == DC - 1),
                                )
                            nc.vector.tensor_copy(v_b[b][:, i, 0:DH], v_ps[:, 0:DH])
                    return f

                out += [qk_chunk(j) for j in range(NJ)]
                out.append(swaps)
                out.append(rot_mul)
                out.append(rot_add)
                out += [v_group(g) for g in range(4)]
                return out

            # ---- phase 1 for batch 0, inline ----
            xt0 = issue_x_dma(0)
            with tc.tile_pool(name="ps_p1", bufs=2, space="PSUM") as ps_p1:
                alloc_ctx["f"] = lambda: ps_p1.tile([P, 512], F32, tag="f", name="fp1")
                b0_work = ph1_bundles(0, xt0)
                for fn in b0_work[:7]:   # qk chunks + swaps + rotary
                    fn()
                nc.sync.dma_start(wv_t[:], wv_d[:, :, :])
                nc.sync.dma_start(v_b[0][:, :, DH : DH + 1], onesc_d[:, :, None])
                for fn in b0_work[7:]:   # v groups
                    fn()
            # ebias first half (jq=0)
            nc.sync.dma_start(ebias_t[:, 0:4, :], ebias_d[:, 0:4, 0:QT])
            nc.sync.dma_start(ebias_t[:, 4:8, :], ebias_d[:, 4:8, 0:QT])
            for b in range(1, B):
                nc.sync.dma_start(v_b[b][:, :, DH : DH + 1], onesc_d[:, :, None])
            xt1 = cp.tile([P, DC, N], BF16, tag="xt", name="xt1", bufs=2)
            nc.sync.dma_start(xt1[:, :, 0 : N // 2], xT_d[1, :, :, 0 : N // 2])
            nc.sync.dma_start(ebias_t[:, 8:12, :], ebias_d[:, 8:12, 0:QT])
            nc.sync.dma_start(xt1[:, :, N // 2 : N], xT_d[1, :, :, N // 2 : N])
            nc.sync.dma_start(ebias_t[:, 12:KC, :], ebias_d[:, 12:KC, 0:QT])
            nc.sync.dma_start(wout_t[0:DH, :], wout_d[:, :])
            nc.sync.dma_start(wout_t[DH:P, :], wout_d[:, :])

            # ---- phase 2 ----
            with (
                tc.tile_pool(name="p2", bufs=1) as p2,
                tc.tile_pool(name="ps_sA", bufs=1, space="PSUM") as ps_sA,
                tc.tile_pool(name="ps_sB", bufs=1, space="PSUM") as ps_sB,
                tc.tile_pool(name="ps_o", bufs=1, space="PSUM") as ps_o,
                tc.tile_pool(name="ps_f", bufs=2, space="PSUM") as ps_f,
            ):
                falloc = lambda: ps_f.tile([P, 512], F32, tag="f", name="ftile")
                alloc_ctx["f"] = falloc
                late_q = deque()

                def drain_late(n=1):
                    for _ in range(n):
                        if late_q:
                            late_q.popleft()()

                tiles = [(b, jq) for jq in range(NQT) for b in range(B)]
                NT = len(tiles)
                TOT = NT * KC

                def make_S(i):
                    t, kc = divmod(i, KC)
                    b, jq = tiles[t]
                    ks = slice(kc * P, (kc + 1) * P)
                    pool = ps_sA if i % 2 == 0 else ps_sB
                    s_ps = pool.tile([P, QT], F32, tag="s", name=f"s_{t}_{kc}")
                    for h in range(QT // 512):
                        qs = slice(jq * QT + h * 512, jq * QT + (h + 1) * 512)
                        nc.tensor.matmul(
                            s_ps[:, h * 512 : (h + 1) * 512],
                            lhsT=kT_b[b][:, ks],
                            rhs=qT_b[b][:, qs],
                            start=True,
                            stop=True,
                        )
                    return s_ps

                et_map = {}      # (t, kc) -> et tile
                outT = {}        # t -> psum [P, NR, 128]
                rec_map = {}     # t -> recip sbuf [P, NR]
                ho_map = {}      # t -> ho sbuf [P, NR, DH]
                hoT_map = {}     # t -> list of 4 hoT tiles [P, P]

                def pv_region(t, r):
                    b, jq = tiles[t]
                    if r == 0:
                        outT[t] = ps_o.tile([P, NR, 128], F32, tag="outT",
                                            name=f"outT{t}")
                    o = outT[t]
                    for kcc in range(KC):
                        nc.tensor.matmul(
                            o[:, r, 0 : DH + 1],
                            lhsT=et_map[(t, kcc)][:, r * P : (r + 1) * P],
                            rhs=v_b[b][:, kcc],
                            start=(kcc == 0),
                            stop=(kcc == KC - 1),
                        )
                    if r == NR - 1:
                        for kcc in range(KC):
                            del et_map[(t, kcc)]
                        rec = cp.tile([P, NR], F32, tag="rec", bufs=2)
                        nc.vector.reciprocal(rec[:], o[:, :, DH])
                        rec_map[t] = rec

                def ho_half(t, half):
                    """Scaled copies of 4 regions + 2 pair-transposes."""
                    o = outT[t]
                    rec = rec_map[t]
                    if half == 0:
                        ho_map[t] = cp.tile([P, NR, DH], BF16, tag="ho", bufs=2,
                                            name=f"ho{t}")
                        hoT_map[t] = []
                    ho = ho_map[t]
                    for r in range(half * 4, half * 4 + 4):
                        nc.vector.scalar_tensor_tensor(
                            ho[:, r, :], o[:, r, 0:DH], rec[:, r : r + 1],
                            ho[:, r, :], op0=MULT, op1=BYP,
                        )
                    for pr in range(half * 2, half * 2 + 2):
                        hoT = cp.tile([P, P], BF16, tag="hoT", bufs=8,
                                      name=f"hoT{t}_{pr}")
                        nc.sync.dma_start_transpose(hoT[:], ho[:, 2 * pr : 2 * pr + 2, :])
                        hoT_map[t].append(hoT)
                    if half == 1:
                        outT.pop(t)
                        rec_map.pop(t)

                def wout_pair(t, pr):
                    """W_out + store for regions 2*pr, 2*pr+1."""
                    b, jq = tiles[t]
                    hoT = hoT_map[t][pr]
                    for rr in range(2):
                        r = 2 * pr + rr
                        wo = falloc()
                        nc.tensor.matmul(
                            wo[:],
                            lhsT=hoT[rr * DH : (rr + 1) * DH, :],
                            rhs=wout_t[rr * DH : (rr + 1) * DH, :],
                            start=True,
                            stop=True,
                        )
                        st = p2.tile([P, DIM], BF16, tag="st", bufs=4)
                        nc.vector.tensor_copy(st[:], wo[:])
                        nc.sync.dma_start(out_d[b, jq * NR + r, :, :], st[:])
                    if pr == 3:
                        ho_map.pop(t)
                        hoT_map.pop(t)

                # prefill
                xt_holder = [None]
                s_map = {}
                for i in range(3):
                    s_map[i] = make_S(i)

                for i in range(TOT):
                    t, kc = divmod(i, KC)
                    b, jq = tiles[t]
                    j = i + 3
                    if j < TOT:
                        s_map[j] = make_S(j)
                    if kc == 0 and jq == 0 and b + 1 < B:
                        for fn in ph1_bundles(b + 1, xt_holder[0] if b else xt1):
                            late_q.append(fn)
                    if kc == 0 and t >= 2:
                        ho_half(t - 2, 0)
                    elif kc == 1 and t >= 2:
                        ho_half(t - 2, 1)

                    # exp (ACT) then in-place bias multiply (DVE or Pool)
                    et = cp.tile([P, QT], BF16, tag="et", bufs=34,
                                 name=f"et{t}_{kc}")
                    nc.scalar.activation(et[:], s_map[i][:], EXP)
                    del s_map[i]
                    eng = nc.gpsimd if kc in POOL_MUL_KCS else nc.vector
                    eng.tensor_mul(et[:], et[:], ebias_t[:, kc, :])
                    et_map[(t, kc)] = et

                    # ebias second-half refresh while the last jq=0 tile runs
                    if t == B - 1 and jq == 0:
                        nc.sync.dma_start(ebias_t[:, kc, :],
                                          ebias_d[:, kc, QT:N])

                    # deferred work for tile t-1 / t-2
                    if t >= 1:
                        if kc in (2, 4, 6, 8, 10, 12, 14):
                            pv_region(t - 1, kc // 2 - 1)
                        elif kc == 15:
                            pv_region(t - 1, 7)
                    if t >= 2:
                        if kc in (3, 5, 7, 9):
                            wout_pair(t - 2, (kc - 3) // 2)

                    if kc in (1, 2, 3, 4, 5):
                        drain_late(2)
                    elif kc in (6, 8, 10, 12):
                        drain_late(1)

                    if kc == 11 and jq == 0 and b + 2 < B:
                        xt_holder[0] = issue_x_dma(b + 2)

                # ---- tail: finish tiles NT-2, NT-1 ----
                while late_q:
                    drain_late(1)
                ho_half(NT - 2, 0)
                ho_half(NT - 2, 1)
                for r in range(NR):
                    pv_region(NT - 1, r)
                    if r >= 4:
                        wout_pair(NT - 2, r - 4)
                ho_half(NT - 1, 0)
                ho_half(NT - 1, 1)
                for pr in range(4):
                    wout_pair(NT - 1, pr)

    nc.compile()
    return nc


def _host_inputs(x, pos_bias, W_qkv, W_out):
    """Build the per-core input maps (pure data marshalling)."""
    bf16 = ml_dtypes.bfloat16
    xT = np.ascontiguousarray(x.transpose(0, 2, 1))               # [B, DIM, N]
    xT4 = np.ascontiguousarray(
        xT.reshape(B, DC, P, N).transpose(0, 2, 1, 3)
    ).astype(bf16)                                                # [B, P, DC, N]

    inv_freq = 1.0 / (10000.0 ** (np.arange(0, DH, 2, dtype=np.float32) / DH))
    freqs = np.arange(N, dtype=np.float32)[:, None] * inv_freq[None, :]  # [N, 32]
    cosT = np.cos(freqs).T.astype(np.float32)                     # [32, N]
    sinT = np.sin(freqs).T.astype(np.float32)
    c4 = np.concatenate([cosT, cosT, cosT, cosT], axis=0).astype(bf16)
    s4 = np.concatenate([-sinT, sinT, -sinT, sinT], axis=0).astype(bf16)

    onesc = np.ones((P, KC), dtype=bf16)

    perm = np.concatenate([np.arange(0, DH, 2), np.arange(1, DH, 2)])

    scale = np.float32(DH ** -0.5)
    in_maps = []
    for h in range(HEADS):
        Wq = W_qkv[:, h * DH : (h + 1) * DH][:, perm] * scale
        Wk = W_qkv[:, DIM + h * DH : DIM + (h + 1) * DH][:, perm]
        Wv = W_qkv[:, 2 * DIM + h * DH : 2 * DIM + (h + 1) * DH]
        Wqk = np.concatenate([Wq, Wk], axis=1).astype(np.float32)  # [512, 128]
        wqk = np.ascontiguousarray(
            Wqk.reshape(DC, P, P).transpose(1, 0, 2)
        ).astype(bf16)                                             # [P, DC, P]
        wv = np.ascontiguousarray(
            Wv.astype(np.float32).reshape(DC, P, DH).transpose(1, 0, 2)
        ).astype(bf16)                                             # [P, DC, DH]
        wout = W_out[h * DH : (h + 1) * DH, :].astype(bf16)        # [64, 512]
        eb = np.exp(pos_bias[h].T).astype(bf16)                    # [2048 k, 2048 q]
        ebias = np.ascontiguousarray(
            eb.reshape(KC, P, N).transpose(1, 0, 2)
        )                                                          # [P, KC, N]
        in_maps.append(
            {
                "xT": xT4,
                "wqk": wqk,
                "wv": wv,
                "wout": wout,
                "c4": c4,
                "s4": s4,
                "ebias": ebias,
                "onesc": onesc,
            }
        )
    return in_maps


def get_nc():
    if "nc" not in _CACHE:
        _CACHE["nc"] = _build()
    return _CACHE["nc"]


def kernel(x, pos_bias, W_qkv, W_out):
    from concourse.bass_utils import run_bass_kernel_spmd

    x = np.asarray(x, dtype=np.float32)
    pos_bias = np.asarray(pos_bias, dtype=np.float32)
    W_qkv = np.asarray(W_qkv, dtype=np.float32)
    W_out = np.asarray(W_out, dtype=np.float32)

    nc = get_nc()
    in_maps = _host_inputs(x, pos_bias, W_qkv, W_out)
    res = run_bass_kernel_spmd(nc, in_maps, core_ids=list(range(HEADS)))
    out = np.zeros((B, N, DIM), dtype=np.float32)
    for rmap in res.results:
        out += rmap["out"].astype(np.float32).reshape(B, N, DIM)
    return out


if __name__ == "__main__":
    rng = np.random.default_rng(0)
    x = rng.standard_normal((B, N, DIM), dtype=np.float32)
    pb = rng.standard_normal((HEADS, N, N), dtype=np.float32)
    wq = rng.standard_normal((DIM, 3 * DIM), dtype=np.float32) * DIM**-0.5
    wo = rng.standard_normal((DIM, DIM), dtype=np.float32) * DIM**-0.5
    o = kernel(x, pb, wq, wo)
    print("kernel ran, out std:", o.std())
